# revision 1
# baseline (speedup 1.0000x reference)
"""AdaptiveSplineLayer on 8 Trainium2 NeuronCores (Bass/Tile).

Reference computation (per element, feature i, row m):
    sort grid[i], mc = (coeffs*sigmoid(alive)) sorted by grid order
    xn = clip((clip(x, gmin, gmax) - gmin) / range * 11, ...)
    spline = lerp of mc at floor(xn)
    out = spline @ proj_w.T + proj_b + x @ res_w.T

Kernel strategy (data-parallel over M; spline params + weights replicated):
  * Host: w = gscale*x + gbias (unclamped normalized coordinate, transposed
    to feature-major). The piecewise-linear spline with uniform knots is
    exactly  f(w) = mc0 + sum_{j=0..11} D_j * relu(w - j)  where the two end
    kinks reproduce the clamping. mc0 and the res-path affine correction
    fold into the output bias; 1/gscale folds into the res weights, so the
    device only ever sees w.
  * Device: 6 fused custom-DVE ops per feature tile evaluate all 12 kinks
    (2 kinks per 8-stage op), writing spline^T in bf16. TensorE contracts
    [spline^T ; w^T] (K=2048) against [proj_w^T ; res_w^T/gscale] in bf16,
    PSUM-accumulated, seeded by a K=1 fp16 matmul that adds the folded bias.
"""

import os
import sys

import numpy as np

for _p in ("/opt/trn_rl_repo",):
    if _p not in sys.path and os.path.isdir(_p):
        sys.path.insert(0, _p)

import ml_dtypes

BF16 = ml_dtypes.bfloat16
F16 = np.float16

M, IN, OUT, K = 16384, 1024, 1024, 12
N_CORES = 8
MC = M // N_CORES  # 2048 rows per core
FT = IN // 128  # 8 feature tiles
MT = MC // 128  # 16 m tiles per core
OC = OUT // 512  # 2 output column chunks
NKINK = K  # 12 kink terms j=0..11

# --------------------------------------------------------------------------
# Custom DVE ops: two relu-kinks per pass.
#   SPLINE_PAIR0   : out = s0*relu(in0-imm2) + s1*relu(in0-(imm2+1))
#   SPLINE_PAIR_ACC: out = in1 + s0*relu(in0-imm2) + s1*relu(in0-(imm2+1))
# --------------------------------------------------------------------------


def _register_spline_ops():
    from concourse.dve_ops import (
        CUSTOM_DVE_SPECS,
        OPS,
        _SUB_OPCODE_FOR_NAME,
        DveOp,
    )
    from concourse.dve_spec import (
        C0,
        C1,
        C2,
        One,
        Spec,
        Src0,
        Src1,
        _has_src1,
        lower,
        relu,
    )
    from concourse.dve_uop import DveOpSpec

    def _dve_relu(x):
        return np.maximum(
            np.nan_to_num(x, nan=0.0, posinf=np.inf, neginf=-np.inf), 0
        )

    def _ref_pair_acc(in0, in1, s0, s1, imm2):
        return (
            in1
            + s0 * _dve_relu(in0.astype(np.float32) - imm2)
            + s1 * _dve_relu(in0.astype(np.float32) - (imm2 + 1.0))
        ).astype(np.float32)

    def _ref_pair0(in0, in1, s0, s1, imm2):
        return (
            s0 * _dve_relu(in0.astype(np.float32) - imm2)
            + s1 * _dve_relu(in0.astype(np.float32) - (imm2 + 1.0))
        ).astype(np.float32)

    def _reg(name, spec):
        if name in _SUB_OPCODE_FOR_NAME:
            return next(o for o in OPS if o.name == name)
        row = max(_SUB_OPCODE_FOR_NAME.values()) + 1
        assert row < 0x20
        op = DveOp.__new__(DveOp)
        object.__setattr__(op, "name", name)
        object.__setattr__(op, "spec", spec)
        object.__setattr__(op, "subdim", False)
        object.__setattr__(op, "perf_en", {})
        s = DveOpSpec(
            name=name,
            opcode=row,
            uops=lower(spec, ver="v3"),
            rd1_en=_has_src1(spec),
        )
        object.__setattr__(op, "uops_sha", {"v3": s.sha("v3")})
        OPS.append(op)
        _SUB_OPCODE_FOR_NAME[name] = row
        CUSTOM_DVE_SPECS[name] = spec
        return op

    body_acc = (Src1 + C0 * relu(Src0 - C2)) + C1 * relu(Src0 - (C2 + One))
    body0 = C0 * relu(Src0 - C2) + C1 * relu(Src0 - (C2 + One))
    pair_acc = _reg("SPLINE_PAIR_ACC", Spec(body=body_acc, reference=_ref_pair_acc))
    pair0 = _reg("SPLINE_PAIR0", Spec(body=body0, reference=_ref_pair0))
    return pair0, pair_acc


# --------------------------------------------------------------------------
# Device graph
# --------------------------------------------------------------------------

_GRAPH_CACHE = {}


def _build_graph(m_split=2):
    key = m_split
    if key in _GRAPH_CACHE:
        return _GRAPH_CACHE[key]

    import concourse.bacc as bacc
    import concourse.mybir as mybir
    import concourse.tile as tile

    pair0, pair_acc = _register_spline_ops()

    dt = mybir.dt
    nc = bacc.Bacc("TRN2", target_bir_lowering=False, debug=False, num_devices=1)

    w32 = nc.dram_tensor("w32", [IN, MC], dt.float32, kind="ExternalInput")
    w16 = nc.dram_tensor("w16", [IN, MC], dt.bfloat16, kind="ExternalInput")
    wt = nc.dram_tensor("wt", [2 * IN, OUT], dt.bfloat16, kind="ExternalInput")
    bias = nc.dram_tensor("bias", [1, OUT], dt.float16, kind="ExternalInput")
    dcoef = nc.dram_tensor("dcoef", [IN, NKINK], dt.float32, kind="ExternalInput")
    out = nc.dram_tensor("out", [MC, OUT], dt.float32, kind="ExternalOutput")

    MS = MC // m_split  # free-dim span per spline op

    with tile.TileContext(nc) as tc:
        with (
            tc.tile_pool(name="const", bufs=1) as const_pool,
            tc.tile_pool(name="wtp", bufs=1) as wt_pool,
            tc.tile_pool(name="w32p", bufs=3) as w32_pool,
            tc.tile_pool(name="persist", bufs=1) as persist_pool,
            tc.tile_pool(name="accp", bufs=3) as acc_pool,
            tc.tile_pool(name="outp", bufs=4) as out_pool,
            tc.tile_pool(name="psum", bufs=8, space="PSUM") as psum_pool,
        ):
            # ---- constants / weights ----
            wt_t = wt_pool.tile([128, 2 * FT, OUT], dt.bfloat16)
            nc.sync.dma_start(wt_t[:], wt.rearrange("(c p) o -> p c o", p=128))
            bias_t = const_pool.tile([1, OUT], dt.float16)
            nc.sync.dma_start(bias_t[:], bias[:])
            ones_t = const_pool.tile([1, 128], dt.float16)
            nc.vector.memset(ones_t[:], 1.0)
            dc_t = const_pool.tile([128, FT, NKINK], dt.float32)
            nc.sync.dma_start(dc_t[:], dcoef.rearrange("(t p) j -> p t j", p=128))

            # persistent bf16 inputs to the matmuls
            w16_t = persist_pool.tile([128, FT, MC], dt.bfloat16, tag="w16")
            spl_t = persist_pool.tile([128, FT, MC], dt.bfloat16, tag="spl")

            # ---- spline evaluation (VectorE custom ops) ----
            for t in range(FT):
                nc.sync.dma_start(
                    w16_t[:, t, :], w16[128 * t : 128 * (t + 1), :]
                )
                w32_tile = w32_pool.tile([128, MC], dt.float32, tag="w32")
                nc.sync.dma_start(
                    w32_tile[:], w32[128 * t : 128 * (t + 1), :]
                )
                dcs = [dc_t[:, t, j : j + 1] for j in range(NKINK)]
                for h in range(m_split):
                    sl = slice(h * MS, (h + 1) * MS)
                    w32_s = w32_tile[:, sl]
                    acc = acc_pool.tile([128, MS], dt.float32, tag="acc")
                    nc.vector._custom_dve(
                        pair0, out=acc[:], in0=w32_s,
                        s0=dcs[0], s1=dcs[1], imm2=0.0,
                    )
                    for q in range(1, NKINK // 2 - 1):
                        nxt = acc_pool.tile([128, MS], dt.float32, tag="acc")
                        nc.vector._custom_dve(
                            pair_acc, out=nxt[:], in0=w32_s, in1=acc[:],
                            s0=dcs[2 * q], s1=dcs[2 * q + 1], imm2=float(2 * q),
                        )
                        acc = nxt
                    nc.vector._custom_dve(
                        pair_acc, out=spl_t[:, t, sl], in0=w32_s, in1=acc[:],
                        s0=dcs[NKINK - 2], s1=dcs[NKINK - 1],
                        imm2=float(NKINK - 2),
                    )

            # ---- matmuls: out[mt, oc] = bias + w^T @ W_res' + spline^T @ proj_w^T
            for mt in range(MT):
                msl = slice(128 * mt, 128 * (mt + 1))
                for oc in range(OC):
                    osl = slice(512 * oc, 512 * (oc + 1))
                    ps = psum_pool.tile([128, 512], dt.float32)
                    nc.tensor.matmul(
                        ps[:], ones_t[:], bias_t[:, osl],
                        start=True, stop=False,
                    )
                    for t in range(FT):
                        nc.tensor.matmul(
                            ps[:], w16_t[:, t, msl], wt_t[:, FT + t, osl],
                            start=False, stop=False,
                        )
                    for t in range(FT):
                        nc.tensor.matmul(
                            ps[:], spl_t[:, t, msl], wt_t[:, t, osl],
                            start=False, stop=(t == FT - 1),
                        )
                    ot = out_pool.tile([128, 512], dt.float32, tag="evac")
                    nc.scalar.copy(ot[:], ps[:])
                    nc.sync.dma_start(out[msl, osl], ot[:])

    nc.compile()
    _GRAPH_CACHE[key] = nc
    return nc


# --------------------------------------------------------------------------
# Host-side parameter preparation
# --------------------------------------------------------------------------


def _prep(x, grid, coeffs, knot_alive, proj_w, proj_b, res_w):
    g64 = grid.astype(np.float64)
    order = np.argsort(g64, axis=1, kind="stable")
    sg = np.take_along_axis(grid.astype(np.float32), order, axis=1)
    # masked coeffs, sorted by grid order (sigmoid in f32 like the reference)
    mcu = coeffs.astype(np.float32) * (
        1.0 / (1.0 + np.exp(-knot_alive.astype(np.float32)))
    )
    mc = np.take_along_axis(mcu, order, axis=1).astype(np.float64)  # (IN, K)

    gmin = sg[:, 0].astype(np.float64)
    gmax = sg[:, -1].astype(np.float64)
    rng = np.maximum(gmax - gmin, 1e-6)
    gscale = (K - 1) / rng  # (IN,)
    gbias = -gmin * gscale

    # kink decomposition: f(w) = mc0 + sum_{j=0..11} D_j relu(w - j)
    s = mc[:, 1:] - mc[:, :-1]  # slopes, (IN, 11)
    D = np.empty((IN, NKINK), dtype=np.float64)
    D[:, 0] = s[:, 0]
    D[:, 1:11] = s[:, 1:] - s[:, :-1]
    D[:, 11] = -s[:, -1]

    # normalized coordinate, feature-major
    w = (
        x.astype(np.float64) * gscale[None, :] + gbias[None, :]
    ).T  # (IN, M) f64
    w32 = np.ascontiguousarray(w, dtype=np.float32)
    w16 = w32.astype(BF16)

    pwT = proj_w.astype(np.float64).T  # (IN, OUT)
    rwT = res_w.astype(np.float64).T  # (IN, OUT)
    w_res = rwT / gscale[:, None]  # res fold: x = (w - gbias)/gscale
    wt = np.concatenate([pwT, w_res], axis=0).astype(BF16)  # (2*IN, OUT)

    bfold = (
        proj_b.astype(np.float64)
        + mc[:, 0] @ pwT
        - gbias @ w_res
    )
    bias = np.ascontiguousarray(bfold[None, :], dtype=F16)

    dcoef = np.ascontiguousarray(D, dtype=np.float32)
    return w32, w16, wt, bias, dcoef


def _make_in_maps(inputs):
    w32, w16, wt, bias, dcoef = _prep(**inputs)
    in_maps = []
    for c in range(N_CORES):
        sl = slice(c * MC, (c + 1) * MC)
        in_maps.append(
            {
                "w32": np.ascontiguousarray(w32[:, sl]),
                "w16": np.ascontiguousarray(w16[:, sl]),
                "wt": wt,
                "bias": bias,
                "dcoef": dcoef,
            }
        )
    return in_maps


def kernel(**inputs):
    from concourse.bass_utils import run_bass_kernel_spmd

    nc = _build_graph()
    in_maps = _make_in_maps(inputs)
    res = run_bass_kernel_spmd(nc, in_maps, core_ids=list(range(N_CORES)))
    return np.concatenate(
        [res.results[c]["out"] for c in range(N_CORES)], axis=0
    )


if __name__ == "__main__":
    rng = np.random.default_rng(0)
    fake = {
        "x": rng.standard_normal((M, IN), dtype=np.float32),
        "grid": rng.standard_normal((IN, K), dtype=np.float32),
        "coeffs": rng.standard_normal((IN, K), dtype=np.float32) * 0.1,
        "knot_alive": rng.standard_normal((IN, K), dtype=np.float32) + 3,
        "proj_w": rng.standard_normal((OUT, IN), dtype=np.float32) / 32,
        "proj_b": rng.standard_normal((OUT,), dtype=np.float32) * 0.01,
        "res_w": rng.standard_normal((OUT, IN), dtype=np.float32) / 32,
    }
    y = kernel(**fake)
    print("kernel output", y.shape, y.dtype)
